# revision 1
# baseline (speedup 1.0000x reference)
"""Paged-attention decode kernel for 8 Trainium2 NeuronCores.

Problem: B=32 decode sequences, GQA (32 q heads / 8 kv heads), head_dim=128,
paged KV cache of 2048 blocks x 16 tokens. Scatter new k/v tokens, then for
each sequence attend over its (up to 2048) cached tokens selected by a block
table.

Strategy:
  - Host: apply the (tiny) slot_mapping scatter, cast caches to bf16, sort
    sequences by length and assign 4 per core (longest spread across cores so
    every core gets rank 0..7, 8..15, ... -> balanced), build per-core int16
    token-slot index lists, softmax validity masks and a pre-transposed,
    pre-scaled qT operand.
  - Device (identical SPMD program on 8 cores): for each of 4 sequence slots,
    gpsimd.dma_gather pulls the sequence's tokens from the bf16 cache:
    K with transpose=True (lands as K^T: [d=128 partitions, kv_head, token]),
    V natural ([token partitions, kv_head*d]). Per kv head: QK^T matmuls
    produce scores^T [128 tokens x 4 q] in PSUM, ScalarE exponentiates
    (no max subtraction needed: |scores| <= ~40 << 88), DVE multiplies by the
    validity mask, then PV matmuls accumulate output [4 x 128] plus a
    ones-column matmul accumulating the softmax denominator into column 128
    of the same PSUM bank. DVE normalizes into the output tile.
"""

import os
import sys
from contextlib import ExitStack

import numpy as np

for _p in ("/opt/trn_rl_repo", "/root/.axon_site/_ro/trn_rl_repo"):
    if os.path.isdir(_p) and _p not in sys.path:
        sys.path.insert(0, _p)

import ml_dtypes  # noqa: E402

import concourse.bass as bass  # noqa: E402
from concourse import bacc  # noqa: E402
import concourse.tile as tile  # noqa: E402
from concourse import mybir  # noqa: E402

B = 32
NUM_BLOCKS = 2048
BLOCK_SIZE = 16
KVH = 8
NH = 32
D = 128
MAX_BLOCKS = 128
G = NH // KVH  # 4 q heads per kv head
ROWS = NUM_BLOCKS * BLOCK_SIZE  # 32768 flat cache rows
ROW_ELEMS = KVH * D  # 1024 elements per token row
SCALE = float(1.0 / np.sqrt(D))
N_CORES = 8
SLOTS = 4  # sequences per core
CG = int(os.environ.get("KRN_CG", "512"))  # tokens per gather chunk-group
NQ = int(os.environ.get("KRN_NQ", "2"))  # SWDGE queues
KVBUFS = int(os.environ.get("KRN_KVBUFS", "8"))  # kt/vt pool buffers
SCRATCH = int(os.environ.get("KRN_SCRATCH", "16384"))  # SWDGE descriptor ring bytes
CHUNK = 128  # tokens per matmul chunk
NCHMAX = CG // CHUNK  # matmul chunks per gather group
D16 = NCHMAX * G  # denominator partial rows
BF16 = mybir.dt.bfloat16
F32 = mybir.dt.float32

_prog_cache: dict = {}


def _build_program(buckets, repeat=1):
    """One SPMD program for all 8 cores; buckets[j] = padded token count of
    sequence slot j (multiple of CHUNK=128, sorted descending).

    repeat > 1 duplicates the whole compute body (same inputs/outputs) for
    marginal-time benchmarking."""
    n_cg = [(b + CG - 1) // CG for b in buckets]  # gather groups (last may be short)
    idx_cols = [b // 16 for b in buckets]
    mask_cols = [(b // CHUNK) * G for b in buckets]
    idx_off = np.cumsum([0] + idx_cols).tolist()
    mask_off = np.cumsum([0] + mask_cols).tolist()
    IDXC = idx_off[-1]
    MC = mask_off[-1]

    nc = bacc.Bacc(num_swdge_queues=NQ, dynamic_dma_scratch_size=SCRATCH)
    kc_d = nc.declare_dram_parameter("kc", [ROWS, ROW_ELEMS], BF16, isOutput=False)
    vc_d = nc.declare_dram_parameter("vc", [ROWS, ROW_ELEMS], BF16, isOutput=False)
    qT_d = nc.declare_dram_parameter("qT", [128, 128], BF16, isOutput=False)
    idx_d = nc.declare_dram_parameter("idx", [128, IDXC], mybir.dt.int16, isOutput=False)
    mask_d = nc.declare_dram_parameter("mask", [128, MC], BF16, isOutput=False)
    sel_d = nc.declare_dram_parameter("sel", [D16, G], F32, isOutput=False)
    out_d = nc.declare_dram_parameter("out", [128, D], F32, isOutput=True)

    with tile.TileContext(nc) as tc, ExitStack() as ctx:
        const = ctx.enter_context(tc.tile_pool(name="const", bufs=1))
        ktp = ctx.enter_context(tc.tile_pool(name="ktp", bufs=KVBUFS))
        vtp = ctx.enter_context(tc.tile_pool(name="vtp", bufs=KVBUFS))
        ptp = ctx.enter_context(tc.tile_pool(name="ptp", bufs=4))
        scp = ctx.enter_context(tc.tile_pool(name="scp", bufs=3, space=bass.MemorySpace.PSUM))
        oap = ctx.enter_context(tc.tile_pool(name="oap", bufs=3, space=bass.MemorySpace.PSUM))
        d16p = ctx.enter_context(tc.tile_pool(name="d16p", bufs=2, space=bass.MemorySpace.PSUM))
        rp = ctx.enter_context(tc.tile_pool(name="rp", bufs=4))
        s16p = ctx.enter_context(tc.tile_pool(name="s16p", bufs=4))
        osbp = ctx.enter_context(tc.tile_pool(name="osbp", bufs=2))

        idx = const.tile([128, IDXC], mybir.dt.int16)
        c0 = min(CG // 16, idx_cols[0])
        nc.sync.dma_start(idx[:, 0:c0], idx_d[:, 0:c0])
        if IDXC > c0:
            nc.sync.dma_start(idx[:, c0:IDXC], idx_d[:, c0:IDXC])
        qT = const.tile([128, 128], BF16)
        nc.sync.dma_start(qT[:], qT_d[:])
        mask = const.tile([128, MC], BF16)
        nc.sync.dma_start(mask[:], mask_d[:])
        sel = const.tile([D16, G], F32)
        nc.sync.dma_start(sel[:], sel_d[:])
        ones = const.tile([128, 1], BF16)
        nc.vector.memset(ones[:], 1.0)

        out_v = out_d.rearrange("(s h g) d -> s g h d", s=SLOTS, h=KVH, g=G)
        for _rep in range(repeat):
         for i in range(SLOTS):
            b = buckets[i]
            kts, vts, sizes = [], [], []
            for cg in range(n_cg[i]):
                toks = min(CG, b - cg * CG)  # multiple of CHUNK
                sizes.append(toks)
                c0 = idx_off[i] + cg * (CG // 16)
                isl = idx[:, c0 : c0 + toks // 16]
                kt = ktp.tile([128, KVH, toks], BF16)
                nc.gpsimd.dma_gather(
                    kt[:], kc_d[:], isl,
                    num_idxs=toks, num_idxs_reg=toks, elem_size=ROW_ELEMS,
                    transpose=True, queue_num=(cg % (NQ // 2)) * 2 if NQ > 2 else 0,
                )
                vt = vtp.tile([128, toks // CHUNK, ROW_ELEMS], BF16)
                nc.gpsimd.dma_gather(
                    vt[:], vc_d[:], isl,
                    num_idxs=toks, num_idxs_reg=toks, elem_size=ROW_ELEMS,
                    transpose=False, queue_num=(cg % (NQ // 2)) * 2 + 1 if NQ > 2 else 1,
                )
                kts.append(kt)
                vts.append(vt)
            o_slot = osbp.tile([G, KVH * D], F32)
            tot = b // CHUNK
            rows16 = min(NCHMAX, tot) * G
            for h in range(KVH):
                qcol = (i * KVH + h) * G
                oa = oap.tile([G, D], F32)
                d16 = d16p.tile([D16, 1], F32)
                nmm = 0
                for cg in range(n_cg[i]):
                    nch = sizes[cg] // CHUNK
                    sc = scp.tile([128, nch * G], F32)
                    for c in range(nch):
                        # scores^T chunk: [128 tokens, G] = K^T_chunk.T @ qT
                        nc.tensor.matmul(
                            sc[:, c * G : (c + 1) * G],
                            kts[cg][:, h, c * CHUNK : (c + 1) * CHUNK],
                            qT[:, qcol : qcol + G],
                            start=True, stop=True,
                        )
                    pt = ptp.tile([128, nch * G], BF16)
                    nc.scalar.activation(pt[:], sc[:], mybir.ActivationFunctionType.Exp)
                    m0 = mask_off[i] + cg * (CG // CHUNK) * G
                    nc.vector.tensor_mul(pt[:], pt[:], mask[:, m0 : m0 + nch * G])
                    for c in range(nch):
                        nc.tensor.matmul(
                            oa[0:G, 0:D],
                            pt[:, c * G : (c + 1) * G],
                            vts[cg][:, c, h * D : (h + 1) * D],
                            start=(nmm == 0), stop=(nmm == tot - 1),
                            skip_group_check=True,
                        )
                        nmm += 1
                    # partial softmax denominators: d16[c*G+g] += sum_s pt[s, c*G+g]
                    nc.tensor.matmul(
                        d16[0 : nch * G, 0:1], pt[:], ones[:],
                        start=(cg == 0), stop=(cg == n_cg[i] - 1),
                        skip_group_check=True,
                    )
                s16 = s16p.tile([D16, 1], F32)
                nc.vector.tensor_copy(s16[0:rows16, :], d16[0:rows16, 0:1])
                # fold chunk-partials per q head: d16[g] = sum_{k%4==g} s16[k]
                nc.tensor.matmul(
                    d16[0:G, 0:1], sel[0:rows16, :], s16[0:rows16, :],
                    start=True, stop=True, skip_group_check=True,
                )
                r = rp.tile([G, 1], F32)
                nc.vector.reciprocal(r[:], d16[0:G, 0:1])
                nc.vector.tensor_scalar_mul(
                    o_slot[0:G, h * D : (h + 1) * D], oa[0:G, 0:D], r[:]
                )
            nc.sync.dma_start(out_v[i], o_slot[:])
    nc.finalize()
    return nc


def _prep(q, k, v, k_cache, v_cache, context_lens, block_tables, slot_mapping):
    """Host-side prep: scatter, bf16 cast, slot assignment, per-core operands."""
    lens = np.asarray(context_lens).astype(np.int64)
    bt = np.asarray(block_tables).astype(np.int64)
    sm = np.asarray(slot_mapping).astype(np.int64)

    kc = np.ascontiguousarray(np.asarray(k_cache, np.float32)).reshape(ROWS, ROW_ELEMS).copy()
    vc = np.ascontiguousarray(np.asarray(v_cache, np.float32)).reshape(ROWS, ROW_ELEMS).copy()
    kc[sm] = np.asarray(k, np.float32).reshape(B, ROW_ELEMS)
    vc[sm] = np.asarray(v, np.float32).reshape(B, ROW_ELEMS)
    kc16 = kc.astype(ml_dtypes.bfloat16)
    vc16 = vc.astype(ml_dtypes.bfloat16)

    order = np.argsort(-lens, kind="stable")
    buckets = tuple(
        max(CHUNK, int(np.ceil(lens[order[j * N_CORES]] / CHUNK)) * CHUNK)
        for j in range(SLOTS)
    )
    IDXC = sum(b // 16 for b in buckets)
    MC = sum((b // CHUNK) * G for b in buckets)

    qs = (np.asarray(q, np.float32)[:, 0] * SCALE).reshape(B, NH, D)
    sel = (np.arange(D16)[:, None] % G == np.arange(G)[None, :]).astype(np.float32)

    in_maps = []
    for n in range(N_CORES):
        qT = np.zeros((128, 128), np.float32)
        idxs = np.zeros((16, IDXC), np.int16)
        msk = np.zeros((128, MC), np.float32)
        io = 0
        mo = 0
        for j in range(SLOTS):
            s = int(order[j * N_CORES + n])
            L = int(lens[s])
            b = buckets[j]
            cols = b // 16
            nb_used = min((L + 15) // 16, cols)
            blocks = np.zeros(cols, np.int64)
            blocks[:nb_used] = bt[s, :nb_used]
            idxs[:, io : io + cols] = (
                blocks[None, :] * BLOCK_SIZE + np.arange(16)[:, None]
            ).astype(np.int16)
            nch = b // CHUNK
            valid = (
                np.arange(128)[:, None] + np.arange(nch)[None, :] * CHUNK
            ) < L  # [128, nch]
            msk[:, mo : mo + nch * G] = np.repeat(valid.astype(np.float32), G, axis=1)
            qT[:, j * 32 : (j + 1) * 32] = qs[s].reshape(32, D).T
            io += cols
            mo += nch * G
        in_maps.append(
            {
                "kc": kc16,
                "vc": vc16,
                "qT": qT.astype(ml_dtypes.bfloat16),
                "idx": np.ascontiguousarray(np.tile(idxs, (8, 1))),
                "mask": msk.astype(ml_dtypes.bfloat16),
                "sel": sel,
            }
        )
    return buckets, order, in_maps


def _assemble(order, core_outs):
    out = np.zeros((B, 1, NH, D), np.float32)
    for n in range(N_CORES):
        o = np.asarray(core_outs[n], np.float32)
        for j in range(SLOTS):
            s = int(order[j * N_CORES + n])
            out[s, 0, :, :] = o[j * 32 : (j + 1) * 32, :]
    return out


def kernel(q, k, v, k_cache, v_cache, context_lens, block_tables, slot_mapping):
    from concourse.bass_utils import run_bass_kernel_spmd

    buckets, order, in_maps = _prep(
        q, k, v, k_cache, v_cache, context_lens, block_tables, slot_mapping
    )
    key = ("hw", buckets)
    if key not in _prog_cache:
        _prog_cache[key] = _build_program(buckets)
    nc = _prog_cache[key]
    res = run_bass_kernel_spmd(nc, in_maps, list(range(N_CORES)))
    return _assemble(order, [res.results[n]["out"] for n in range(N_CORES)])



# revision 15
# speedup vs baseline: 1.6913x; 1.6913x over previous
"""Paged-attention decode kernel for 8 Trainium2 NeuronCores.

Problem: B=32 decode sequences, GQA (32 q heads / 8 kv heads), head_dim=128,
paged KV cache of 2048 blocks x 16 tokens. Scatter new k/v tokens, then for
each sequence attend over its (up to 2048) cached tokens selected by a block
table.

v2 strategy (v1 in kernel_v1.py):
  - Host: apply the slot_mapping scatter, cast caches to bf16, sort sequences
    into 4 rank blocks of 8 and greedily balance blocks across cores so each
    core's total gathered tokens is near sum/8 (the v1 rank-rigid assignment
    left a 4832 vs 3072 token imbalance). Build per-core idx lists padded
    with -1 tails, per-slot token counts, softmax masks, pre-scaled qT.
  - Device: per slot ONE K gather + ONE V gather (num_idxs=2048 static,
    num_idxs_reg loaded at runtime from the counts input), so only the
    actual ceil16(L) tokens move and SWDGE descriptor-generation (994ns fixed
    per gather on the gpsimd engine) drops from 22 to 8 instructions/rep.
    Pool buffers are memset once at start so ungathered tails are zeros.
    Per (slot, head): QK^T chunk matmuls -> scores^T [tok, G] in PSUM,
    exp (no max subtraction: |scores| <= ~40 << 88), mask multiply, then
    PV^T matmuls accumulate o^T [d=128, G] (4-col streams instead of the v1
    128-col o[G, d] streams) and a ones-lhsT matmul accumulates softmax
    denominators into a [32, G] PSUM tile. Normalization happens on host
    (out is o^T and den, both un-normalized).
"""

import os
import sys
from contextlib import ExitStack

import numpy as np

for _p in ("/opt/trn_rl_repo", "/root/.axon_site/_ro/trn_rl_repo"):
    if os.path.isdir(_p) and _p not in sys.path:
        sys.path.insert(0, _p)

import ml_dtypes  # noqa: E402

import concourse.bass as bass  # noqa: E402
from concourse import bacc  # noqa: E402
import concourse.tile as tile  # noqa: E402
from concourse import mybir  # noqa: E402

B = 32
NUM_BLOCKS = 2048
BLOCK_SIZE = 16
KVH = 8
NH = 32
D = 128
MAX_BLOCKS = 128
G = NH // KVH  # 4 q heads per kv head
ROWS = NUM_BLOCKS * BLOCK_SIZE  # 32768 flat cache rows
ROW_ELEMS = KVH * D  # 1024 elements per token row
SCALE = float(1.0 / np.sqrt(D))
N_CORES = 8
SLOTS = 4  # sequences per core
NQ = int(os.environ.get("KRN_NQ", "2"))  # SWDGE queues
KVBUFS = int(os.environ.get("KRN_KVBUFS", "8"))  # kt pool buffers
SCRATCH = int(os.environ.get("KRN_SCRATCH", "16384"))  # SWDGE descriptor ring
CHUNK = 128  # tokens per matmul chunk
STATIC = os.environ.get("KRN_STATIC", "0") == "1"  # debug: no runtime regs
SMAX = MAX_BLOCKS * BLOCK_SIZE  # 2048 max tokens per slot
WINK = int(os.environ.get("KRN_WINK", "512"))  # tokens per K gather window
WINV = int(os.environ.get("KRN_WINV", "512"))  # tokens per V gather window
VBUFS = int(os.environ.get("KRN_VBUFS", "8"))  # vt pool buffers
BF16 = mybir.dt.bfloat16
F32 = mybir.dt.float32

_prog_cache: dict = {}


def _build_program(buckets, repeat=1, mode="full"):
    """One SPMD program for all 8 cores; buckets[j] = padded token count of
    sequence slot j (multiple of CHUNK=128, sorted descending). Gathers are
    runtime-sized from the "cnt" input; compute is static over buckets.

    repeat > 1 duplicates the compute body for marginal-time benchmarking.
    mode: "full" | "gather" (no compute) | "compute" (host limits counts)."""
    nch = [b // CHUNK for b in buckets]
    nwk = [(b + WINK - 1) // WINK for b in buckets]
    nwv = [(b + WINV - 1) // WINV for b in buckets]
    wkoff = np.cumsum([0] + nwk).tolist()
    wvoff = np.cumsum([0] + nwv).tolist()
    NW = wkoff[-1] + wvoff[-1]
    mask_off = np.cumsum([0] + [n * G for n in nch]).tolist()
    MC = mask_off[-1]

    nc = bacc.Bacc(num_swdge_queues=NQ, dynamic_dma_scratch_size=SCRATCH)
    kc_d = nc.declare_dram_parameter("kc", [ROWS, ROW_ELEMS], BF16, isOutput=False)
    vc_d = nc.declare_dram_parameter("vc", [ROWS, ROW_ELEMS], BF16, isOutput=False)
    qT_d = nc.declare_dram_parameter("qT", [128, 128], BF16, isOutput=False)
    idx_d = nc.declare_dram_parameter(
        "idx", [128, SLOTS * (SMAX // 16)], mybir.dt.int16, isOutput=False
    )
    mask_d = nc.declare_dram_parameter("mask", [128, MC], BF16, isOutput=False)
    cnt_d = nc.declare_dram_parameter("cnt", [1, NW], mybir.dt.int32, isOutput=False)
    oT_d = nc.declare_dram_parameter("oT", [128, 128], F32, isOutput=True)
    den_d = nc.declare_dram_parameter("den", [1, SLOTS * KVH * G], F32, isOutput=True)

    with tile.TileContext(nc) as tc, ExitStack() as ctx:
        const = ctx.enter_context(tc.tile_pool(name="const", bufs=1))
        ktp = ctx.enter_context(tc.tile_pool(name="ktp", bufs=KVBUFS))
        vtp = ctx.enter_context(tc.tile_pool(name="vtp", bufs=VBUFS))
        ptp = ctx.enter_context(tc.tile_pool(name="ptp", bufs=4))
        scp = ctx.enter_context(tc.tile_pool(name="scp", bufs=3, space=bass.MemorySpace.PSUM))
        oap = ctx.enter_context(tc.tile_pool(name="oap", bufs=2, space=bass.MemorySpace.PSUM))
        dnp = ctx.enter_context(tc.tile_pool(name="dnp", bufs=2, space=bass.MemorySpace.PSUM))
        osbp = ctx.enter_context(tc.tile_pool(name="osbp", bufs=2))

        idx = const.tile([128, SLOTS * (SMAX // 16)], mybir.dt.int16)
        nc.sync.dma_start(idx[:], idx_d[:])
        qT = const.tile([128, 128], BF16)
        nc.sync.dma_start(qT[:], qT_d[:])
        mask = const.tile([128, MC], BF16)
        nc.sync.dma_start(mask[:], mask_d[:])
        cntt = const.tile([1, NW], mybir.dt.int32)
        nc.sync.dma_start(cntt[:], cnt_d[:])
        ones = const.tile([128, 1], BF16)
        nc.vector.memset(ones[:], 1.0)

        regs = [
            nc.values_load(
                cntt[0:1, w : w + 1],
                engines=[mybir.EngineType.Pool],
                skip_runtime_bounds_check=True,
            )
            for w in range(NW)
        ]

        for _rep in range(repeat):
            if mode != "gather":
                oT = oap.tile([128, 128], F32)
                den = dnp.tile([1, SLOTS * KVH * G], F32)
            oT_sb = osbp.tile([128, 256], F32)
            for j in range(SLOTS):
                kts, vts = [], []
                for w in range(nwk[j]):
                    toks = min(WINK, buckets[j] - w * WINK)
                    i0 = j * (SMAX // 16) + w * (WINK // 16)
                    isl = idx[:, i0 : i0 + toks // 16]
                    reg = toks if STATIC else regs[wkoff[j] + w]
                    kt = ktp.tile([128, KVH, toks], BF16)
                    if _rep == 0:
                        # Zero once so runtime-short gathers leave exact
                        # zeros (exp(0)*mask0 = 0; PV on zeros adds 0).
                        nc.vector.memset(kt[:], 0.0)
                    nc.gpsimd.dma_gather(
                        kt[:], kc_d[:], isl,
                        num_idxs=toks, num_idxs_reg=reg, elem_size=ROW_ELEMS,
                        transpose=True, queue_num=0,
                    )
                    kts.append(kt)
                for w in range(nwv[j]):
                    toks = min(WINV, buckets[j] - w * WINV)
                    i0 = j * (SMAX // 16) + w * (WINV // 16)
                    isl = idx[:, i0 : i0 + toks // 16]
                    reg = toks if STATIC else regs[wkoff[-1] + wvoff[j] + w]
                    vt = vtp.tile([128, toks // CHUNK, ROW_ELEMS], BF16)
                    if _rep == 0:
                        nc.vector.memset(vt[:], 0.0)
                    nc.gpsimd.dma_gather(
                        vt[:], vc_d[:], isl,
                        num_idxs=toks, num_idxs_reg=reg, elem_size=ROW_ELEMS,
                        transpose=False, queue_num=1 % NQ,
                    )
                    vts.append(vt)
                if mode == "gather":
                    continue
                WCK = WINK // CHUNK
                WCV = WINV // CHUNK
                for h in range(KVH):
                    row = j * KVH + h
                    qcol = row * G
                    sc = scp.tile([128, nch[j] * G], F32)
                    for c in range(nch[j]):
                        nc.tensor.matmul(
                            sc[:, c * G : (c + 1) * G],
                            kts[c // WCK][:, h, (c % WCK) * CHUNK : (c % WCK + 1) * CHUNK],
                            qT[:, qcol : qcol + G],
                            start=True, stop=True,
                        )
                    pt = ptp.tile([128, nch[j] * G], BF16)
                    nc.scalar.activation(pt[:], sc[:], mybir.ActivationFunctionType.Exp)
                    m0 = mask_off[j]
                    nc.vector.tensor_mul(pt[:], pt[:], mask[:, m0 : m0 + nch[j] * G])
                    for c in range(nch[j]):
                        nc.tensor.matmul(
                            oT[:, qcol : qcol + G],
                            vts[c // WCV][:, c % WCV, h * D : (h + 1) * D],
                            pt[:, c * G : (c + 1) * G],
                            start=(c == 0), stop=(c == nch[j] - 1),
                            skip_group_check=True,
                        )
                        nc.tensor.matmul(
                            den[0:1, qcol : qcol + G],
                            ones[:],
                            pt[:, c * G : (c + 1) * G],
                            start=(c == 0), stop=(c == nch[j] - 1),
                            skip_group_check=True,
                        )
            if mode == "gather":
                nc.vector.memset(oT_sb[:], 0.0)
                nc.sync.dma_start(oT_d[:], oT_sb[:, 0:128])
                nc.sync.dma_start(den_d[:], oT_sb[0:1, 128:256])
                continue
            nc.vector.tensor_copy(oT_sb[:, 0:128], oT[:])
            nc.vector.tensor_copy(oT_sb[0:1, 128:256], den[:])
            nc.sync.dma_start(oT_d[:], oT_sb[:, 0:128])
            nc.sync.dma_start(den_d[:], oT_sb[0:1, 128:256])
    nc.finalize()
    return nc


def _prep(q, k, v, k_cache, v_cache, context_lens, block_tables, slot_mapping):
    """Host-side prep: scatter, bf16 cast, balanced slot assignment, per-core
    operands (idx with -1 tails, runtime counts, masks, qT)."""
    lens = np.asarray(context_lens).astype(np.int64)
    bt = np.asarray(block_tables).astype(np.int64)
    sm = np.asarray(slot_mapping).astype(np.int64)

    kc = np.ascontiguousarray(np.asarray(k_cache, np.float32)).reshape(ROWS, ROW_ELEMS).copy()
    vc = np.ascontiguousarray(np.asarray(v_cache, np.float32)).reshape(ROWS, ROW_ELEMS).copy()
    kc[sm] = np.asarray(k, np.float32).reshape(B, ROW_ELEMS)
    vc[sm] = np.asarray(v, np.float32).reshape(B, ROW_ELEMS)
    kc16 = kc.astype(ml_dtypes.bfloat16)
    vc16 = vc.astype(ml_dtypes.bfloat16)

    srt = np.argsort(-lens, kind="stable")
    # Rank blocks of 8 keep the static buckets minimal; within each block,
    # give the longest remaining sequence to the least-loaded core.
    order = np.zeros(B, np.int64)  # order[j*N_CORES + n] = seq of core n slot j
    load = np.zeros(N_CORES, np.int64)
    for j in range(SLOTS):
        block = list(srt[j * N_CORES : (j + 1) * N_CORES])  # sorted desc
        free = set(range(N_CORES))
        for s in block:
            n = min(free, key=lambda c: (load[c], c))
            free.remove(n)
            order[j * N_CORES + n] = s
            load[n] += (lens[s] + 15) // 16 * 16
    buckets = tuple(
        max(CHUNK, int(np.ceil(lens[srt[j * N_CORES]] / CHUNK)) * CHUNK)
        for j in range(SLOTS)
    )
    nch = [b // CHUNK for b in buckets]
    nwk = [(b + WINK - 1) // WINK for b in buckets]
    nwv = [(b + WINV - 1) // WINV for b in buckets]
    wkoff = np.cumsum([0] + nwk).tolist()
    wvoff = np.cumsum([0] + nwv).tolist()
    NW = wkoff[-1] + wvoff[-1]
    MC = sum(n * G for n in nch)

    qs = (np.asarray(q, np.float32)[:, 0] * SCALE).reshape(B, NH, D)

    in_maps = []
    for n in range(N_CORES):
        qT = np.zeros((128, 128), np.float32)
        idxs = np.full((16, SLOTS * (SMAX // 16)), 0 if STATIC else -1, np.int16)
        msk = np.zeros((128, MC), np.float32)
        cnt = np.zeros((1, NW), np.int32)
        mo = 0
        for j in range(SLOTS):
            s = int(order[j * N_CORES + n])
            L = int(lens[s])
            c16 = (L + 15) // 16 * 16
            nb = c16 // 16
            io = j * (SMAX // 16)
            idxs[:, io : io + nb] = (
                bt[s, :nb][None, :] * BLOCK_SIZE + np.arange(16)[:, None]
            ).astype(np.int16)
            for w in range(nwk[j]):
                toks = min(WINK, buckets[j] - w * WINK)
                lo = w * WINK
                c_w = min(max(c16 - lo, 16), toks)
                cnt[0, wkoff[j] + w] = c_w
                if c16 - lo < 16:
                    # ensure >=16 valid entries per window (safe idx 0,
                    # masked out) so num_idxs_reg is never 0
                    idxs[:, io + lo // 16] = 0
            for w in range(nwv[j]):
                toks = min(WINV, buckets[j] - w * WINV)
                lo = w * WINV
                cnt[0, wkoff[-1] + wvoff[j] + w] = min(max(c16 - lo, 16), toks)
            valid = (
                np.arange(128)[:, None] + np.arange(nch[j])[None, :] * CHUNK
            ) < L  # [128, nch]
            msk[:, mo : mo + nch[j] * G] = np.repeat(valid.astype(np.float32), G, axis=1)
            qT[:, j * 32 : (j + 1) * 32] = qs[s].reshape(32, D).T
            mo += nch[j] * G
        in_maps.append(
            {
                "kc": kc16,
                "vc": vc16,
                "qT": qT.astype(ml_dtypes.bfloat16),
                "idx": np.ascontiguousarray(np.tile(idxs, (8, 1))),
                "mask": msk.astype(ml_dtypes.bfloat16),
                "cnt": cnt,
            }
        )
    return buckets, order, in_maps


def _assemble(order, core_outs):
    out = np.zeros((B, 1, NH, D), np.float32)
    for n in range(N_CORES):
        oT = np.asarray(core_outs[n]["oT"], np.float32)  # [128, (j,h,g)]
        den = np.asarray(core_outs[n]["den"], np.float32).reshape(-1)  # (j,h,g)
        for j in range(SLOTS):
            s = int(order[j * N_CORES + n])
            blk = oT[:, j * 32 : (j + 1) * 32]  # [d, (h,g)]
            d_blk = den[j * 32 : (j + 1) * 32]  # (h,g)
            out[s, 0, :, :] = (blk / d_blk[None, :]).T
    return out


def kernel(q, k, v, k_cache, v_cache, context_lens, block_tables, slot_mapping):
    from concourse.bass_utils import run_bass_kernel_spmd

    buckets, order, in_maps = _prep(
        q, k, v, k_cache, v_cache, context_lens, block_tables, slot_mapping
    )
    key = ("hw", buckets)
    if key not in _prog_cache:
        _prog_cache[key] = _build_program(buckets)
    nc = _prog_cache[key]
    res = run_bass_kernel_spmd(nc, in_maps, list(range(N_CORES)))
    return _assemble(order, res.results)
